# revision 10
# baseline (speedup 1.0000x reference)
"""Distributed causal-attention kernel for 8 Trainium2 NeuronCores.

Reference computation (B=2, T=2048, C=2048, H=16, hd=128):
  q,k,v = rope(x @ Wq.T), rope(x @ Wk.T), x @ Wv.T   (per-head)
  y = (softmax(q k^T / sqrt(hd) + mask) v, concat heads) @ Wo.T

Sharding: tensor-parallel over heads across all 8 cores (H/8 heads per
core, both batches processed on every core). Per-head attention runs in
the transposed layout (S^T = k_tile^T q_chunk) so the PV matmul needs
no transposes; softmax skips the max-subtraction (scores are bounded
here, exp stays in fp32 range) and gets its denominator via a
ones-vector matmul (partition-axis sum). A single 8-core AllToAll then
hands core (b*4+g) the full set of heads for batch b, t-slice g, and
each core computes that slice's o_proj. Matmuls run in float32r (full
PE rate; measured numerically identical to the fp32 matmul path on
TRN2).
"""
import sys

sys.path.insert(0, '/opt/trn_rl_repo')

import numpy as np
import concourse.bass as bass
import concourse.bacc as bacc
import concourse.mybir as mybir
import concourse.tile as tile
from concourse import bass_utils

F32 = mybir.dt.float32
F32R = mybir.dt.float32r
AF = mybir.ActivationFunctionType

ROPE_BASE = 10000.0
HD = 128           # head dim (C // n_heads)
B = 2              # batch (fixed: cores 0-3 <-> b=0, 4-7 <-> b=1)
N_CORES = 8


def _rope_tables(T):
    """Transposed RoPE tables [hd, T] plus the sign-folded sin table.

    q' = q * cosT + qswap * sinT_signed, where qswap is q with its
    partition halves swapped (no sign change):
      rows d < 64:  q'[d] = q[d] cos[d] - q[d+64] sin[d]
      rows d >= 64: q'[d] = q[d] cos[d] + q[d-64] sin[d]  (sin[d]=sin[d-64])
    """
    inv_freq = 1.0 / (ROPE_BASE ** (np.arange(0, HD, 2, dtype=np.float64) / HD))
    t = np.arange(T, dtype=np.float64)
    freqs = np.outer(t, inv_freq)                      # [T, hd/2]
    emb = np.concatenate([freqs, freqs], -1)           # [T, hd]
    cos = np.cos(emb).T.astype(np.float32)             # [hd, T]
    sin = np.sin(emb).T.astype(np.float32)
    sin_signed = sin.copy()
    sin_signed[:HD // 2] *= -1.0
    return cos, sin_signed


def _causal_binmask():
    """Wide binary mask [128, 896]: W[kk, i] = 1 iff i >= kk + 384.

    For a diagonal S^T tile with k-tile offset o in 0..3 relative to the
    512-wide q-chunk start, slice [:, 384-128*o : 896-128*o] gives
    keep[kk, qq] = (qq >= kk + 128*o)  i.e. k_global <= q_global.
    """
    kk = np.arange(128)[:, None]
    i = np.arange(896)[None, :]
    return (i >= kk + 384).astype(np.float32)


def build_nc(T, C, mode="causal"):
    """Build the SPMD Bass program. All 8 cores run identical code;
    per-core behavior (which heads / which output slice) comes from the
    inputs and the AllToAll.

    mode: 'causal' (skip upper-triangle blocks, binary-mask diagonal),
          'full' (no masking), 'masked' (additive mask input).
    """
    HPC = C // HD // N_CORES     # heads per core
    D = HPC * HD                 # local channel count
    NCT = C // 128               # contraction tiles over C
    TO = T // 4                  # output t-slice width per core
    NQC = T // 512               # 512-wide q chunks per batch
    TCH = 256                    # projection t chunk
    NCH = B * T // TCH           # projection chunks (both batches)
    NTT = T // 128               # k/t tiles per batch
    scale = 1.0 / np.sqrt(HD)

    nc = bacc.Bacc("TRN2", target_bir_lowering=False, debug=False,
                   num_devices=N_CORES)

    # x of both batches, transposed: [C, 2T] = [x[0].T | x[1].T]
    xT = nc.dram_tensor("xT", [C, B * T], F32R, kind="ExternalInput")
    wqT = nc.dram_tensor("wqT", [C, D], F32R, kind="ExternalInput")
    wkT = nc.dram_tensor("wkT", [C, D], F32R, kind="ExternalInput")
    wvT = nc.dram_tensor("wvT", [C, D], F32R, kind="ExternalInput")
    woT = nc.dram_tensor("woT", [C, C], F32R, kind="ExternalInput")
    cosT = nc.dram_tensor("cosT", [HD, T], F32, kind="ExternalInput")
    sinT = nc.dram_tensor("sinT", [HD, T], F32, kind="ExternalInput")
    ones_in = nc.dram_tensor("ones_in", [128, 1], F32R, kind="ExternalInput")
    onesr_in = nc.dram_tensor("onesr_in", [1, 128], F32R, kind="ExternalInput")
    if mode == "causal":
        bmask = nc.dram_tensor("bmask", [128, 896], F32, kind="ExternalInput")
    elif mode == "masked":
        maskT = nc.dram_tensor("maskT", [T, T], F32, kind="ExternalInput")
    y = nc.dram_tensor("y", [TO, C], F32, kind="ExternalOutput")

    def ktmax(qj):  # number of k-tiles for q-chunk qj
        return 4 * qj + 4 if mode == "causal" else NTT

    with tile.TileContext(nc) as tc, \
         tc.tile_pool(name="consts", bufs=1) as pc, \
         tc.tile_pool(name="dram", bufs=1, space="DRAM") as dram:
        ones_sb = pc.tile([128, 1], F32R)
        onesr_sb = pc.tile([1, 128], F32R)
        nc.sync.dma_start(ones_sb[:], ones_in[:])
        nc.sync.dma_start(onesr_sb[:], onesr_in[:])
        if mode == "causal":
            bm_sb = pc.tile([128, 896], F32)
            nc.sync.dma_start(bm_sb[:], bmask[:])

        # AllToAll buffers: shard j (rows j*D..(j+1)*D) holds this
        # core's head outputs for dest core j = (b*4 + t-slice).
        # After A2A, cc_out row block i = core i's heads (channels
        # i*D..(i+1)*D), all for THIS core's (batch, t-slice).
        # Split into NSPLIT column-halves so the second A2A overlaps
        # the first half's o_proj.
        NSPLIT = 2 if TO >= 256 else 1
        TH = TO // NSPLIT
        cc_in = [dram.tile([N_CORES * D, TH], F32R, name=f"cc_in{i}")
                 for i in range(NSPLIT)]
        cc_out = [dram.tile([N_CORES * D, TH], F32R, name=f"cc_out{i}")
                  for i in range(NSPLIT)]

        # qkv pool lives through attention, freed before o_proj
        with tc.tile_pool(name="qkv", bufs=1) as pq:
            # per (batch, head) pair p = b*HPC + h: q^T,k^T [hd, T]
            qT_sb = pq.tile([128, B * HPC * T], F32R)
            kT_sb = pq.tile([128, B * HPC * T], F32R)
            # v natural layout per batch: [T,128]-tile x [128, D]
            v_sb = pq.tile([128, B * NTT * D], F32R)

            # ============ Phase A: q,k,v projections + RoPE ============
            with tc.tile_pool(name="prj_w", bufs=1) as pw, \
                 tc.tile_pool(name="prj_x", bufs=2) as px, \
                 tc.tile_pool(name="prj_cs", bufs=2) as pcs, \
                 tc.tile_pool(name="prj_ps", bufs=6, space="PSUM") as pps, \
                 tc.tile_pool(name="prj_psv", bufs=2, space="PSUM") as ppsv, \
                 tc.tile_pool(name="prj_tmp", bufs=2) as pt:
                wq_sb = pw.tile([128, NCT * D], F32R)
                wk_sb = pw.tile([128, NCT * D], F32R)
                wv_sb = pw.tile([128, NCT * D], F32R)
                for wsb, wdr in ((wq_sb, wqT), (wk_sb, wkT), (wv_sb, wvT)):
                    nc.sync.dma_start(
                        wsb.rearrange("p (n d) -> p n d", n=NCT),
                        wdr.rearrange("(n p) d -> p n d", p=128))
                for ch in range(NCH):
                    b = ch // (T // TCH)
                    tloc = (ch * TCH) % T
                    xch = px.tile([128, NCT * TCH], F32R, tag="xch",
                                  name="xch")
                    nc.sync.dma_start(
                        xch.rearrange("p (n t) -> p n t", n=NCT),
                        xT[:, ch * TCH:(ch + 1) * TCH].rearrange(
                            "(n p) t -> p n t", p=128))
                    cs = pcs.tile([128, TCH], F32, tag="cos", name="cs")
                    sn = pcs.tile([128, TCH], F32, tag="sin", name="sn")
                    nc.sync.dma_start(cs[:], cosT[:, tloc:tloc + TCH])
                    nc.sync.dma_start(sn[:], sinT[:, tloc:tloc + TCH])
                    # q,k for each local head
                    for h in range(HPC):
                        p = b * HPC + h
                        for wsb, dst, nm in ((wq_sb, qT_sb, "q"),
                                             (wk_sb, kT_sb, "k")):
                            ps = pps.tile([128, TCH], F32, tag="pAqk",
                                          name="psA")
                            for ct in range(NCT):
                                nc.tensor.matmul(
                                    ps[:],
                                    wsb[:, ct * D + h * HD:
                                        ct * D + (h + 1) * HD],
                                    xch[:, ct * TCH:(ct + 1) * TCH],
                                    start=(ct == 0), stop=(ct == NCT - 1))
                            sl = dst[:, p * T + tloc: p * T + tloc + TCH]
                            tmp = pt.tile([128, TCH], F32, tag="rtmp",
                                          name="rtmp")
                            nc.scalar.copy(tmp[:], ps[:])
                            sw = pt.tile([128, TCH], F32, tag="rsw",
                                         name="rsw")
                            nc.sync.dma_start(sw[0:64, :], tmp[64:128, :])
                            nc.sync.dma_start(sw[64:128, :], tmp[0:64, :])
                            t1 = pt.tile([128, TCH], F32, tag="rt1",
                                         name="t1")
                            nc.vector.tensor_mul(t1[:], ps[:], cs[:])
                            t2 = pt.tile([128, TCH], F32, tag="rt2",
                                         name="t2")
                            nc.vector.tensor_mul(t2[:], sw[:], sn[:])
                            with nc.allow_low_precision(reason="f32r rope"):
                                nc.vector.tensor_add(sl, t1[:], t2[:])
                    # v for this chunk (all local heads at once)
                    for st in range(TCH // 128):
                        tt = (ch * TCH) // 128 + st   # global tile in [0,B*NTT)
                        ps = ppsv.tile([128, D], F32, tag="pV", name="psV")
                        for ct in range(NCT):
                            nc.tensor.matmul(
                                ps[:],
                                xch[:, ct * TCH + st * 128:
                                    ct * TCH + st * 128 + 128],
                                wv_sb[:, ct * D:(ct + 1) * D],
                                start=(ct == 0), stop=(ct == NCT - 1))
                        with nc.allow_low_precision(reason="f32r v evac"):
                            nc.scalar.copy(v_sb[:, tt * D:(tt + 1) * D],
                                           ps[:])

            # ============ Attention per (batch, head) =================
            # Software-pipelined emission: the S^T matmul for k-tile
            # kt+2 is issued before the den/PV matmuls of k-tile kt, so
            # the PE keeps streaming while ACT(exp)/DVE(mask) catch up.
            with tc.tile_pool(name="att_es", bufs=6) as pes, \
                 tc.tile_pool(name="att_o", bufs=3) as po, \
                 tc.tile_pool(name="att_ps", bufs=3, space="PSUM") as pas, \
                 tc.tile_pool(name="att_acc", bufs=2, space="PSUM") as paa, \
                 tc.tile_pool(name="att_msk", bufs=4) as pmk:
                for b in range(B):
                    for h in range(HPC):
                        p = b * HPC + h
                        for qj in range(NQC):
                            qsl = qT_sb[:, p * T + qj * 512:
                                        p * T + qj * 512 + 512]
                            kmax = ktmax(qj)
                            ps_den = paa.tile([1, 512], F32, tag="den",
                                              name="psden")
                            ps_o = paa.tile([128, 512], F32, tag="pvacc",
                                            name="pso")

                            def s_mm(kt):
                                ps_s = pas.tile([128, 512], F32, tag="s",
                                                name="pss")
                                nc.tensor.matmul(
                                    ps_s[:],
                                    kT_sb[:, p * T + kt * 128:
                                          p * T + kt * 128 + 128],
                                    qsl, start=True, stop=True)
                                return ps_s

                            s_tiles = {0: s_mm(0)}
                            if kmax > 1:
                                s_tiles[1] = s_mm(1)
                            for kt in range(kmax):
                                ps_s = s_tiles.pop(kt)
                                if mode == "masked":
                                    sm = pmk.tile([128, 512], F32, tag="sm",
                                                  name="sm")
                                    mt = pmk.tile([128, 512], F32, tag="mt",
                                                  name="mt")
                                    nc.sync.dma_start(
                                        mt[:],
                                        maskT[kt * 128:(kt + 1) * 128,
                                              qj * 512:(qj + 1) * 512])
                                    nc.vector.tensor_add(sm[:], ps_s[:],
                                                         mt[:])
                                    src = sm
                                else:
                                    src = ps_s
                                e_t = pes.tile([128, 512], F32R, tag="es",
                                               name="et")
                                with nc.allow_low_precision(reason="exp"):
                                    nc.scalar.activation(
                                        e_t[:], src[:], AF.Exp,
                                        scale=float(scale))
                                if mode == "causal" and kt >= 4 * qj:
                                    o = kt - 4 * qj
                                    em = pes.tile([128, 512], F32R,
                                                  tag="esm", name="em")
                                    with nc.allow_low_precision(reason="mask"):
                                        nc.vector.tensor_mul(
                                            em[:], e_t.bitcast(F32),
                                            bm_sb[:, 384 - 128 * o:
                                                  896 - 128 * o])
                                    e_t = em
                                if kt + 2 < kmax:
                                    s_tiles[kt + 2] = s_mm(kt + 2)
                                nc.tensor.matmul(
                                    ps_den[:], ones_sb[:], e_t[:],
                                    start=(kt == 0), stop=(kt == kmax - 1))
                                nc.tensor.matmul(
                                    ps_o[:],
                                    v_sb[:, (b * NTT + kt) * D + h * HD:
                                         (b * NTT + kt) * D + (h + 1) * HD],
                                    e_t[:],
                                    start=(kt == 0), stop=(kt == kmax - 1))
                            rd = po.tile([1, 512], F32R, tag="rd", name="rd")
                            with nc.allow_low_precision(reason="recip"):
                                nc.vector.reciprocal(rd[:], ps_den[:])
                            ps_b = pas.tile([128, 512], F32, tag="bc",
                                            bufs=1, name="psb")
                            nc.tensor.matmul(ps_b[:], onesr_sb[:], rd[:],
                                             start=True, stop=True)
                            o_tmp = po.tile([128, 512], F32, tag="otmp",
                                            name="otmp")
                            nc.vector.tensor_copy(o_tmp[:], ps_o[:])
                            o_sc = po.tile([128, 512], F32R, tag="osc",
                                           name="osc")
                            with nc.allow_low_precision(reason="scale"):
                                nc.vector.tensor_mul(o_sc[:], o_tmp[:],
                                                     ps_b[:])
                            # scatter the 512-wide q-chunk into the A2A
                            # shard/half grid
                            w = min(512, TH)
                            for s in range(512 // w):
                                t0 = qj * 512 + s * w    # global t in batch
                                shard = b * 4 + t0 // TO
                                half = (t0 % TO) // TH
                                nc.sync.dma_start(
                                    cc_in[half][shard * D + h * HD:
                                                shard * D + (h + 1) * HD,
                                                t0 % TH: t0 % TH + w],
                                    o_sc[:, s * w:(s + 1) * w])

        # ============ AllToAll (split for o_proj overlap) =============
        for i in range(NSPLIT):
            nc.gpsimd.collective_compute(
                "AllToAll", mybir.AluOpType.bypass,
                replica_groups=[list(range(N_CORES))],
                ins=[cc_in[i].opt()], outs=[cc_out[i].opt()])

        # ============ Phase C: o_proj for this core's slice ===========
        with tc.tile_pool(name="phC_cc", bufs=1) as pcc, \
             tc.tile_pool(name="phC_w", bufs=2) as pcw, \
             tc.tile_pool(name="phC_y", bufs=4) as pcy, \
             tc.tile_pool(name="phC_ps", bufs=4, space="PSUM") as pcps:
            cc_sb = []
            for i in range(NSPLIT):
                t = pcc.tile([128, NCT * TH], F32R, name=f"cc_sb{i}")
                nc.sync.dma_start(
                    t.rearrange("p (n t) -> p n t", n=NCT),
                    cc_out[i].opt().rearrange("(n p) t -> p n t", p=128))
                cc_sb.append(t)
            for dj in range(C // 512):
                wo_sb = pcw.tile([128, NCT * 512], F32R, tag="wo", name="wo")
                nc.sync.dma_start(
                    wo_sb.rearrange("p (n d) -> p n d", n=NCT),
                    woT[:, dj * 512:(dj + 1) * 512].rearrange(
                        "(n p) d -> p n d", p=128))
                for tt in range(TO // 128):
                    half, tloc = divmod(tt * 128, TH)
                    ps = pcps.tile([128, 512], F32, tag="pC", name="psC")
                    for ct in range(NCT):
                        nc.tensor.matmul(
                            ps[:],
                            cc_sb[half][:, ct * TH + tloc:
                                        ct * TH + tloc + 128],
                            wo_sb[:, ct * 512:(ct + 1) * 512],
                            start=(ct == 0), stop=(ct == NCT - 1))
                    yt = pcy.tile([128, 512], F32, tag="yt", name="yt")
                    nc.scalar.copy(yt[:], ps[:])
                    nc.sync.dma_start(
                        y[tt * 128:(tt + 1) * 128, dj * 512:(dj + 1) * 512],
                        yt[:])

    nc.compile()
    return nc


_NC_CACHE = {}


def _get_nc(T, C, mode):
    key = (T, C, mode)
    if key not in _NC_CACHE:
        _NC_CACHE[key] = build_nc(T, C, mode)
    return _NC_CACHE[key]


def _detect_mode(mask):
    T = mask.shape[0]
    tri = np.tril(np.ones((T, T), dtype=bool))
    if not np.any(mask):
        return "full"
    if np.all(np.abs(mask[tri]) < 1e-6) and np.all(mask[~tri] < -1e8):
        return "causal"
    return "masked"


def kernel(x, mask, Wq, Wk, Wv, Wo):
    x = np.asarray(x)
    mask = np.asarray(mask)
    Bx, T, C = x.shape
    assert Bx == B
    HPC = C // HD // N_CORES
    TO = T // 4
    mode = _detect_mode(mask)
    nc = _get_nc(T, C, mode)

    cos, sin_signed = _rope_tables(T)
    xT2 = np.concatenate([x[0].T, x[1].T], axis=1)
    xT2 = np.ascontiguousarray(xT2)
    in_maps = []
    for core in range(N_CORES):
        hsl = slice(core * HPC * HD, (core + 1) * HPC * HD)
        m = {
            "xT": xT2,
            "wqT": np.ascontiguousarray(np.asarray(Wq)[hsl, :].T),
            "wkT": np.ascontiguousarray(np.asarray(Wk)[hsl, :].T),
            "wvT": np.ascontiguousarray(np.asarray(Wv)[hsl, :].T),
            "woT": np.ascontiguousarray(np.asarray(Wo).T),
            "cosT": cos, "sinT": sin_signed,
            "ones_in": np.ones((128, 1), np.float32),
            "onesr_in": np.ones((1, 128), np.float32),
        }
        if mode == "causal":
            m["bmask"] = _causal_binmask()
        elif mode == "masked":
            m["maskT"] = np.ascontiguousarray(mask.T) * np.float32(np.sqrt(HD))
        in_maps.append(m)

    res = bass_utils.run_bass_kernel_spmd(nc, in_maps,
                                          core_ids=list(range(N_CORES)))

    out = np.empty((B, T, C), np.float32)
    for core in range(N_CORES):
        b, g = divmod(core, 4)
        out[b, g * TO:(g + 1) * TO, :] = res.results[core]["y"]
    return out


# revision 16
# speedup vs baseline: 1.1201x; 1.1201x over previous
"""Distributed causal-attention kernel for 8 Trainium2 NeuronCores.

Reference computation (B=2, T=2048, C=2048, H=16, hd=128):
  q,k,v = rope(x @ Wq.T), rope(x @ Wk.T), x @ Wv.T   (per-head)
  y = (softmax(q k^T / sqrt(hd) + mask) v, concat heads) @ Wo.T

Sharding: tensor-parallel over heads across all 8 cores (H/8 heads per
core, both batches processed on every core). Per-head attention runs in
the transposed layout (S^T = k_tile^T q_chunk) so the PV matmul needs
no transposes; softmax skips the max-subtraction (scores are bounded
here, exp stays in fp32 range) and gets its denominator via a
ones-vector matmul (partition-axis sum). A single 8-core AllToAll then
hands core (b*4+g) the full set of heads for batch b, t-slice g, and
each core computes that slice's o_proj. Matmuls run in float32r (full
PE rate; measured numerically identical to the fp32 matmul path on
TRN2).
"""
import sys

sys.path.insert(0, '/opt/trn_rl_repo')

import numpy as np
import concourse.bass as bass
import concourse.bacc as bacc
import concourse.mybir as mybir
import concourse.tile as tile
from concourse import bass_utils

F32 = mybir.dt.float32
F32R = mybir.dt.float32r
AF = mybir.ActivationFunctionType

ROPE_BASE = 10000.0
HD = 128           # head dim (C // n_heads)
B = 2              # batch (fixed: cores 0-3 <-> b=0, 4-7 <-> b=1)
N_CORES = 8


def _rope_tables(T):
    """Transposed RoPE tables [hd, T] plus the sign-folded sin table.

    q' = q * cosT + qswap * sinT_signed, where qswap is q with its
    partition halves swapped (no sign change):
      rows d < 64:  q'[d] = q[d] cos[d] - q[d+64] sin[d]
      rows d >= 64: q'[d] = q[d] cos[d] + q[d-64] sin[d]  (sin[d]=sin[d-64])
    """
    inv_freq = 1.0 / (ROPE_BASE ** (np.arange(0, HD, 2, dtype=np.float64) / HD))
    t = np.arange(T, dtype=np.float64)
    freqs = np.outer(t, inv_freq)                      # [T, hd/2]
    emb = np.concatenate([freqs, freqs], -1)           # [T, hd]
    cos = np.cos(emb).T.astype(np.float32)             # [hd, T]
    sin = np.sin(emb).T.astype(np.float32)
    sin_signed = sin.copy()
    sin_signed[:HD // 2] *= -1.0
    return cos, sin_signed


def _causal_binmask():
    """Wide binary mask [128, 896]: W[kk, i] = 1 iff i >= kk + 384.

    For a diagonal S^T tile with k-tile offset o in 0..3 relative to the
    512-wide q-chunk start, slice [:, 384-128*o : 896-128*o] gives
    keep[kk, qq] = (qq >= kk + 128*o)  i.e. k_global <= q_global.
    """
    kk = np.arange(128)[:, None]
    i = np.arange(896)[None, :]
    return (i >= kk + 384).astype(np.float32)


def build_nc(T, C, mode="causal"):
    """Build the SPMD Bass program. All 8 cores run identical code;
    per-core behavior (which heads / which output slice) comes from the
    inputs and the AllToAll.

    mode: 'causal' (skip upper-triangle blocks, binary-mask diagonal),
          'full' (no masking), 'masked' (additive mask input).
    """
    HPC = C // HD // N_CORES     # heads per core
    D = HPC * HD                 # local channel count
    NCT = C // 128               # contraction tiles over C
    TO = T // 4                  # output t-slice width per core
    NQC = T // 512               # 512-wide q chunks per batch
    TCH = 256                    # projection t chunk
    NCH = B * T // TCH           # projection chunks (both batches)
    NTT = T // 128               # k/t tiles per batch
    scale = 1.0 / np.sqrt(HD)

    nc = bacc.Bacc("TRN2", target_bir_lowering=False, debug=False,
                   num_devices=N_CORES)

    # x of both batches, transposed: [C, 2T] = [x[0].T | x[1].T]
    xT = nc.dram_tensor("xT", [C, B * T], F32R, kind="ExternalInput")
    wqT = nc.dram_tensor("wqT", [C, D], F32R, kind="ExternalInput")
    wkT = nc.dram_tensor("wkT", [C, D], F32R, kind="ExternalInput")
    wvT = nc.dram_tensor("wvT", [C, D], F32R, kind="ExternalInput")
    woT = nc.dram_tensor("woT", [C, C], F32R, kind="ExternalInput")
    cosT = nc.dram_tensor("cosT", [HD, T], F32, kind="ExternalInput")
    sinT = nc.dram_tensor("sinT", [HD, T], F32, kind="ExternalInput")
    ones_in = nc.dram_tensor("ones_in", [128, 1], F32R, kind="ExternalInput")
    onesr_in = nc.dram_tensor("onesr_in", [1, 128], F32R, kind="ExternalInput")
    if mode == "causal":
        bmask = nc.dram_tensor("bmask", [128, 896], F32, kind="ExternalInput")
    elif mode == "masked":
        maskT = nc.dram_tensor("maskT", [T, T], F32, kind="ExternalInput")
    y = nc.dram_tensor("y", [TO, C], F32, kind="ExternalOutput")

    def ktmax(qj):  # number of k-tiles for q-chunk qj
        return 4 * qj + 4 if mode == "causal" else NTT

    with tile.TileContext(nc) as tc, \
         tc.tile_pool(name="consts", bufs=1) as pc, \
         tc.tile_pool(name="dram", bufs=1, space="DRAM") as dram:
        ones_sb = pc.tile([128, 1], F32R)
        onesr_sb = pc.tile([1, 128], F32R)
        nc.sync.dma_start(ones_sb[:], ones_in[:])
        nc.sync.dma_start(onesr_sb[:], onesr_in[:])
        if mode == "causal":
            bm_sb = pc.tile([128, 896], F32)
            nc.sync.dma_start(bm_sb[:], bmask[:])

        # AllToAll buffers, one pair per local head h: shard j (rows
        # j*HD..(j+1)*HD) holds this core's head-h outputs for dest
        # core j = (b*4 + t-slice). After A2A, cc_out[h] row block i =
        # core i's head h (channels i*D + h*HD ..), for THIS core's
        # (batch, t-slice). Per-head tensors let head h's A2A overlap
        # head h+1's attention.
        cc_in = [dram.tile([N_CORES * HD, TO], F32R, name=f"cc_in{h}")
                 for h in range(HPC)]
        cc_out = [dram.tile([N_CORES * HD, TO], F32R, name=f"cc_out{h}")
                  for h in range(HPC)]

        # qkv pool lives through attention, freed before o_proj
        with tc.tile_pool(name="qkv", bufs=1) as pq:
            # per (batch, head) pair p = b*HPC + h: q^T,k^T [hd, T]
            qT_sb = pq.tile([128, B * HPC * T], F32R)
            kT_sb = pq.tile([128, B * HPC * T], F32R)
            # v natural layout per batch: [T,128]-tile x [128, D]
            v_sb = pq.tile([128, B * NTT * D], F32R)

            # ============ Phase A: q,k,v projections + RoPE ============
            with tc.tile_pool(name="prj_w", bufs=1) as pw, \
                 tc.tile_pool(name="prj_x", bufs=2) as px, \
                 tc.tile_pool(name="prj_cs", bufs=2) as pcs, \
                 tc.tile_pool(name="prj_ps", bufs=6, space="PSUM") as pps, \
                 tc.tile_pool(name="prj_psv", bufs=2, space="PSUM") as ppsv, \
                 tc.tile_pool(name="prj_tmp", bufs=2) as pt:
                wq_sb = pw.tile([128, NCT * D], F32R)
                wk_sb = pw.tile([128, NCT * D], F32R)
                wv_sb = pw.tile([128, NCT * D], F32R)
                for wsb, wdr in ((wq_sb, wqT), (wk_sb, wkT), (wv_sb, wvT)):
                    nc.sync.dma_start(
                        wsb.rearrange("p (n d) -> p n d", n=NCT),
                        wdr.rearrange("(n p) d -> p n d", p=128))
                for ch in range(NCH):
                    b = ch // (T // TCH)
                    tloc = (ch * TCH) % T
                    xch = px.tile([128, NCT * TCH], F32R, tag="xch",
                                  name="xch")
                    nc.gpsimd.dma_start(
                        xch.rearrange("p (n t) -> p n t", n=NCT),
                        xT[:, ch * TCH:(ch + 1) * TCH].rearrange(
                            "(n p) t -> p n t", p=128))
                    cs = pcs.tile([128, TCH], F32, tag="cos", name="cs")
                    sn = pcs.tile([128, TCH], F32, tag="sin", name="sn")
                    nc.sync.dma_start(cs[:], cosT[:, tloc:tloc + TCH])
                    nc.sync.dma_start(sn[:], sinT[:, tloc:tloc + TCH])
                    # q,k for each local head
                    for h in range(HPC):
                        p = b * HPC + h
                        for wsb, dst, nm in ((wq_sb, qT_sb, "q"),
                                             (wk_sb, kT_sb, "k")):
                            ps = pps.tile([128, TCH], F32, tag="pAqk",
                                          name="psA")
                            for ct in range(NCT):
                                nc.tensor.matmul(
                                    ps[:],
                                    wsb[:, ct * D + h * HD:
                                        ct * D + (h + 1) * HD],
                                    xch[:, ct * TCH:(ct + 1) * TCH],
                                    start=(ct == 0), stop=(ct == NCT - 1))
                            sl = dst[:, p * T + tloc: p * T + tloc + TCH]
                            tmp = pt.tile([128, TCH], F32, tag="rtmp",
                                          name="rtmp")
                            nc.scalar.copy(tmp[:], ps[:])
                            sw = pt.tile([128, TCH], F32, tag="rsw",
                                         name="rsw")
                            nc.sync.dma_start(sw[0:64, :], tmp[64:128, :])
                            nc.sync.dma_start(sw[64:128, :], tmp[0:64, :])
                            t1 = pt.tile([128, TCH], F32, tag="rt1",
                                         name="t1")
                            nc.vector.tensor_mul(t1[:], ps[:], cs[:])
                            t2 = pt.tile([128, TCH], F32, tag="rt2",
                                         name="t2")
                            nc.vector.tensor_mul(t2[:], sw[:], sn[:])
                            with nc.allow_low_precision(reason="f32r rope"):
                                nc.vector.tensor_add(sl, t1[:], t2[:])
                    # v for this chunk (all local heads at once)
                    for st in range(TCH // 128):
                        tt = (ch * TCH) // 128 + st   # global tile in [0,B*NTT)
                        ps = ppsv.tile([128, D], F32, tag="pV", name="psV")
                        for ct in range(NCT):
                            nc.tensor.matmul(
                                ps[:],
                                xch[:, ct * TCH + st * 128:
                                    ct * TCH + st * 128 + 128],
                                wv_sb[:, ct * D:(ct + 1) * D],
                                start=(ct == 0), stop=(ct == NCT - 1))
                        with nc.allow_low_precision(reason="f32r v evac"):
                            nc.scalar.copy(v_sb[:, tt * D:(tt + 1) * D],
                                           ps[:])

            # ============ Attention per (batch, head) =================
            # Software-pipelined emission: the S^T matmul for k-tile
            # kt+2 is issued before the den/PV matmuls of k-tile kt, so
            # the PE keeps streaming while ACT(exp)/DVE(mask) catch up.
            with tc.tile_pool(name="att_es", bufs=6) as pes, \
                 tc.tile_pool(name="att_o", bufs=3) as po, \
                 tc.tile_pool(name="att_ps", bufs=3, space="PSUM") as pas, \
                 tc.tile_pool(name="att_acc", bufs=2, space="PSUM") as paa, \
                 tc.tile_pool(name="att_msk", bufs=4) as pmk:
                for h in range(HPC):
                    for b in range(B):
                        p = b * HPC + h
                        for qj in range(NQC):
                            qsl = qT_sb[:, p * T + qj * 512:
                                        p * T + qj * 512 + 512]
                            kmax = ktmax(qj)
                            ps_den = paa.tile([1, 512], F32, tag="den",
                                              name="psden")
                            ps_o = paa.tile([128, 512], F32, tag="pvacc",
                                            name="pso")

                            def s_mm(kt):
                                ps_s = pas.tile([128, 512], F32, tag="s",
                                                name="pss")
                                nc.tensor.matmul(
                                    ps_s[:],
                                    kT_sb[:, p * T + kt * 128:
                                          p * T + kt * 128 + 128],
                                    qsl, start=True, stop=True)
                                return ps_s

                            s_tiles = {0: s_mm(0)}
                            if kmax > 1:
                                s_tiles[1] = s_mm(1)
                            for kt in range(kmax):
                                ps_s = s_tiles.pop(kt)
                                if mode == "masked":
                                    sm = pmk.tile([128, 512], F32, tag="sm",
                                                  name="sm")
                                    mt = pmk.tile([128, 512], F32, tag="mt",
                                                  name="mt")
                                    nc.sync.dma_start(
                                        mt[:],
                                        maskT[kt * 128:(kt + 1) * 128,
                                              qj * 512:(qj + 1) * 512])
                                    nc.vector.tensor_add(sm[:], ps_s[:],
                                                         mt[:])
                                    src = sm
                                else:
                                    src = ps_s
                                e_t = pes.tile([128, 512], F32R, tag="es",
                                               name="et")
                                with nc.allow_low_precision(reason="exp"):
                                    nc.scalar.activation(
                                        e_t[:], src[:], AF.Exp,
                                        scale=float(scale))
                                if mode == "causal" and kt >= 4 * qj:
                                    o = kt - 4 * qj
                                    em = pes.tile([128, 512], F32R,
                                                  tag="esm", name="em")
                                    with nc.allow_low_precision(reason="mask"):
                                        nc.vector.tensor_mul(
                                            em[:], e_t.bitcast(F32),
                                            bm_sb[:, 384 - 128 * o:
                                                  896 - 128 * o])
                                    e_t = em
                                if kt + 2 < kmax:
                                    s_tiles[kt + 2] = s_mm(kt + 2)
                                nc.tensor.matmul(
                                    ps_den[:], ones_sb[:], e_t[:],
                                    start=(kt == 0), stop=(kt == kmax - 1))
                                nc.tensor.matmul(
                                    ps_o[:],
                                    v_sb[:, (b * NTT + kt) * D + h * HD:
                                         (b * NTT + kt) * D + (h + 1) * HD],
                                    e_t[:],
                                    start=(kt == 0), stop=(kt == kmax - 1))
                            rd = po.tile([1, 512], F32R, tag="rd", name="rd")
                            with nc.allow_low_precision(reason="recip"):
                                nc.vector.reciprocal(rd[:], ps_den[:])
                            ps_b = pas.tile([128, 512], F32, tag="bc",
                                            bufs=1, name="psb")
                            nc.tensor.matmul(ps_b[:], onesr_sb[:], rd[:],
                                             start=True, stop=True)
                            o_tmp = po.tile([128, 512], F32, tag="otmp",
                                            name="otmp")
                            nc.vector.tensor_copy(o_tmp[:], ps_o[:])
                            o_sc = po.tile([128, 512], F32R, tag="osc",
                                           name="osc")
                            with nc.allow_low_precision(reason="scale"):
                                nc.vector.tensor_mul(o_sc[:], o_tmp[:],
                                                     ps_b[:])
                            # scatter the 512-wide q-chunk into shards
                            w = min(512, TO)
                            for s in range(512 // w):
                                t0 = qj * 512 + s * w    # global t in batch
                                shard = b * 4 + t0 // TO
                                nc.sync.dma_start(
                                    cc_in[h][shard * HD:(shard + 1) * HD,
                                             t0 % TO: t0 % TO + w],
                                    o_sc[:, s * w:(s + 1) * w])
                    # head h complete on both batches -> its AllToAll can
                    # overlap head h+1's attention
                    nc.gpsimd.collective_compute(
                        "AllToAll", mybir.AluOpType.bypass,
                        replica_groups=[list(range(N_CORES))],
                        ins=[cc_in[h].opt()], outs=[cc_out[h].opt()])

        # ============ Phase C: o_proj for this core's slice ===========
        with tc.tile_pool(name="phC_cc", bufs=1) as pcc, \
             tc.tile_pool(name="phC_w", bufs=2) as pcw, \
             tc.tile_pool(name="phC_y", bufs=4) as pcy, \
             tc.tile_pool(name="phC_ps", bufs=4, space="PSUM") as pcps:
            cc_sb = []
            for h in range(HPC):
                t = pcc.tile([128, N_CORES * TO], F32R, name=f"cc_sb{h}")
                nc.sync.dma_start(
                    t.rearrange("p (n t) -> p n t", n=N_CORES),
                    cc_out[h].opt().rearrange("(n p) t -> p n t", p=128))
                cc_sb.append(t)
            for dj in range(C // 512):
                wo_sb = pcw.tile([128, NCT * 512], F32R, tag="wo", name="wo")
                nc.sync.dma_start(
                    wo_sb.rearrange("p (n d) -> p n d", n=NCT),
                    woT[:, dj * 512:(dj + 1) * 512].rearrange(
                        "(n p) d -> p n d", p=128))
                for tt in range(TO // 128):
                    ps = pcps.tile([128, 512], F32, tag="pC", name="psC")
                    for ct in range(NCT):
                        # channel-tile ct = core (ct // HPC), head (ct % HPC)
                        i, hh = divmod(ct, HPC)
                        nc.tensor.matmul(
                            ps[:],
                            cc_sb[hh][:, i * TO + tt * 128:
                                      i * TO + tt * 128 + 128],
                            wo_sb[:, ct * 512:(ct + 1) * 512],
                            start=(ct == 0), stop=(ct == NCT - 1))
                    yt = pcy.tile([128, 512], F32, tag="yt", name="yt")
                    nc.scalar.copy(yt[:], ps[:])
                    nc.sync.dma_start(
                        y[tt * 128:(tt + 1) * 128, dj * 512:(dj + 1) * 512],
                        yt[:])

    nc.compile()
    return nc


_NC_CACHE = {}


def _get_nc(T, C, mode):
    key = (T, C, mode)
    if key not in _NC_CACHE:
        _NC_CACHE[key] = build_nc(T, C, mode)
    return _NC_CACHE[key]


def _detect_mode(mask):
    T = mask.shape[0]
    tri = np.tril(np.ones((T, T), dtype=bool))
    if not np.any(mask):
        return "full"
    if np.all(np.abs(mask[tri]) < 1e-6) and np.all(mask[~tri] < -1e8):
        return "causal"
    return "masked"


def kernel(x, mask, Wq, Wk, Wv, Wo):
    x = np.asarray(x)
    mask = np.asarray(mask)
    Bx, T, C = x.shape
    assert Bx == B
    HPC = C // HD // N_CORES
    TO = T // 4
    mode = _detect_mode(mask)
    nc = _get_nc(T, C, mode)

    cos, sin_signed = _rope_tables(T)
    xT2 = np.concatenate([x[0].T, x[1].T], axis=1)
    xT2 = np.ascontiguousarray(xT2)
    in_maps = []
    for core in range(N_CORES):
        hsl = slice(core * HPC * HD, (core + 1) * HPC * HD)
        m = {
            "xT": xT2,
            "wqT": np.ascontiguousarray(np.asarray(Wq)[hsl, :].T),
            "wkT": np.ascontiguousarray(np.asarray(Wk)[hsl, :].T),
            "wvT": np.ascontiguousarray(np.asarray(Wv)[hsl, :].T),
            "woT": np.ascontiguousarray(np.asarray(Wo).T),
            "cosT": cos, "sinT": sin_signed,
            "ones_in": np.ones((128, 1), np.float32),
            "onesr_in": np.ones((1, 128), np.float32),
        }
        if mode == "causal":
            m["bmask"] = _causal_binmask()
        elif mode == "masked":
            m["maskT"] = np.ascontiguousarray(mask.T) * np.float32(np.sqrt(HD))
        in_maps.append(m)

    res = bass_utils.run_bass_kernel_spmd(nc, in_maps,
                                          core_ids=list(range(N_CORES)))

    out = np.empty((B, T, C), np.float32)
    for core in range(N_CORES):
        b, g = divmod(core, 4)
        out[b, g * TO:(g + 1) * TO, :] = res.results[core]["y"]
    return out


# revision 17
# speedup vs baseline: 1.1832x; 1.0564x over previous
"""Distributed causal-attention kernel for 8 Trainium2 NeuronCores.

Reference computation (B=2, T=2048, C=2048, H=16, hd=128):
  q,k,v = rope(x @ Wq.T), rope(x @ Wk.T), x @ Wv.T   (per-head)
  y = (softmax(q k^T / sqrt(hd) + mask) v, concat heads) @ Wo.T

Sharding: tensor-parallel over heads across all 8 cores (H/8 heads per
core, both batches processed on every core). Per-head attention runs in
the transposed layout (S^T = k_tile^T q_chunk) so the PV matmul needs
no transposes; softmax skips the max-subtraction (scores are bounded
here, exp stays in fp32 range) and gets its denominator via a
ones-vector matmul (partition-axis sum). A single 8-core AllToAll then
hands core (b*4+g) the full set of heads for batch b, t-slice g, and
each core computes that slice's o_proj. Matmuls run in float32r (full
PE rate; measured numerically identical to the fp32 matmul path on
TRN2).
"""
import sys

sys.path.insert(0, '/opt/trn_rl_repo')

import numpy as np
import concourse.bass as bass
import concourse.bacc as bacc
import concourse.mybir as mybir
import concourse.tile as tile
from concourse import bass_utils

F32 = mybir.dt.float32
F32R = mybir.dt.float32r
AF = mybir.ActivationFunctionType

ROPE_BASE = 10000.0
HD = 128           # head dim (C // n_heads)
B = 2              # batch (fixed: cores 0-3 <-> b=0, 4-7 <-> b=1)
N_CORES = 8


def _rope_tables(T):
    """Transposed RoPE tables [hd, T] plus the sign-folded sin table.

    q' = q * cosT + qswap * sinT_signed, where qswap is q with its
    partition halves swapped (no sign change):
      rows d < 64:  q'[d] = q[d] cos[d] - q[d+64] sin[d]
      rows d >= 64: q'[d] = q[d] cos[d] + q[d-64] sin[d]  (sin[d]=sin[d-64])
    """
    inv_freq = 1.0 / (ROPE_BASE ** (np.arange(0, HD, 2, dtype=np.float64) / HD))
    t = np.arange(T, dtype=np.float64)
    freqs = np.outer(t, inv_freq)                      # [T, hd/2]
    emb = np.concatenate([freqs, freqs], -1)           # [T, hd]
    cos = np.cos(emb).T.astype(np.float32)             # [hd, T]
    sin = np.sin(emb).T.astype(np.float32)
    sin_signed = sin.copy()
    sin_signed[:HD // 2] *= -1.0
    return cos, sin_signed


def _causal_binmask():
    """Wide binary mask [128, 896]: W[kk, i] = 1 iff i >= kk + 384.

    For a diagonal S^T tile with k-tile offset o in 0..3 relative to the
    512-wide q-chunk start, slice [:, 384-128*o : 896-128*o] gives
    keep[kk, qq] = (qq >= kk + 128*o)  i.e. k_global <= q_global.
    """
    kk = np.arange(128)[:, None]
    i = np.arange(896)[None, :]
    return (i >= kk + 384).astype(np.float32)


def build_nc(T, C, mode="causal"):
    """Build the SPMD Bass program. All 8 cores run identical code;
    per-core behavior (which heads / which output slice) comes from the
    inputs and the AllToAll.

    mode: 'causal' (skip upper-triangle blocks, binary-mask diagonal),
          'full' (no masking), 'masked' (additive mask input).
    """
    HPC = C // HD // N_CORES     # heads per core
    D = HPC * HD                 # local channel count
    NCT = C // 128               # contraction tiles over C
    TO = T // 4                  # output t-slice width per core
    NQC = T // 512               # 512-wide q chunks per batch
    TCH = 256                    # projection t chunk
    NCH = B * T // TCH           # projection chunks (both batches)
    NTT = T // 128               # k/t tiles per batch
    scale = 1.0 / np.sqrt(HD)

    nc = bacc.Bacc("TRN2", target_bir_lowering=False, debug=False,
                   num_devices=N_CORES)

    # x of both batches, transposed: [C, 2T] = [x[0].T | x[1].T]
    xT = nc.dram_tensor("xT", [C, B * T], F32R, kind="ExternalInput")
    wqT = nc.dram_tensor("wqT", [C, D], F32R, kind="ExternalInput")
    wkT = nc.dram_tensor("wkT", [C, D], F32R, kind="ExternalInput")
    wvT = nc.dram_tensor("wvT", [C, D], F32R, kind="ExternalInput")
    woT = nc.dram_tensor("woT", [C, C], F32R, kind="ExternalInput")
    cosT = nc.dram_tensor("cosT", [HD, T], F32, kind="ExternalInput")
    sinT = nc.dram_tensor("sinT", [HD, T], F32, kind="ExternalInput")
    ones_in = nc.dram_tensor("ones_in", [128, 1], F32R, kind="ExternalInput")
    onesr_in = nc.dram_tensor("onesr_in", [1, 128], F32R, kind="ExternalInput")
    if mode == "causal":
        bmask = nc.dram_tensor("bmask", [128, 896], F32, kind="ExternalInput")
    elif mode == "masked":
        maskT = nc.dram_tensor("maskT", [T, T], F32, kind="ExternalInput")
    y = nc.dram_tensor("y", [TO, C], F32, kind="ExternalOutput")

    def ktmax(qj):  # number of k-tiles for q-chunk qj
        return 4 * qj + 4 if mode == "causal" else NTT

    with tile.TileContext(nc) as tc, \
         tc.tile_pool(name="consts", bufs=1) as pc, \
         tc.tile_pool(name="dram", bufs=1, space="DRAM") as dram:
        ones_sb = pc.tile([128, 1], F32R)
        onesr_sb = pc.tile([1, 128], F32R)
        nc.sync.dma_start(ones_sb[:], ones_in[:])
        nc.sync.dma_start(onesr_sb[:], onesr_in[:])
        if mode == "causal":
            bm_sb = pc.tile([128, 896], F32)
            nc.sync.dma_start(bm_sb[:], bmask[:])

        # AllToAll buffers, one pair per local head h: shard j (rows
        # j*HD..(j+1)*HD) holds this core's head-h outputs for dest
        # core j = (b*4 + t-slice). After A2A, cc_out[h] row block i =
        # core i's head h (channels i*D + h*HD ..), for THIS core's
        # (batch, t-slice). Per-head tensors let head h's A2A overlap
        # head h+1's attention.
        cc_in = [dram.tile([N_CORES * HD, TO], F32R, name=f"cc_in{h}")
                 for h in range(HPC)]
        cc_out = [dram.tile([N_CORES * HD, TO], F32R, name=f"cc_out{h}")
                  for h in range(HPC)]

        # qkv pool lives through attention, freed before o_proj
        with tc.tile_pool(name="qkv", bufs=1) as pq:
            # per (batch, head) pair p = b*HPC + h: q^T,k^T [hd, T]
            qT_sb = pq.tile([128, B * HPC * T], F32R)
            kT_sb = pq.tile([128, B * HPC * T], F32R)
            # v natural layout per batch: [T,128]-tile x [128, D]
            v_sb = pq.tile([128, B * NTT * D], F32R)

            # ============ Phase A: q,k,v projections + RoPE ============
            with tc.tile_pool(name="prj_w", bufs=1) as pw, \
                 tc.tile_pool(name="prj_x", bufs=2) as px, \
                 tc.tile_pool(name="prj_cs", bufs=2) as pcs, \
                 tc.tile_pool(name="prj_ps", bufs=8, space="PSUM") as pps, \
                 tc.tile_pool(name="prj_tmp", bufs=2) as pt:
                wq_sb = pw.tile([128, NCT * D], F32R)
                wk_sb = pw.tile([128, NCT * D], F32R)
                wv_sb = pw.tile([128, NCT * D], F32R)
                for wsb, wdr in ((wq_sb, wqT), (wk_sb, wkT), (wv_sb, wvT)):
                    nc.sync.dma_start(
                        wsb.rearrange("p (n d) -> p n d", n=NCT),
                        wdr.rearrange("(n p) d -> p n d", p=128))
                for ch in range(NCH):
                    b = ch // (T // TCH)
                    tloc = (ch * TCH) % T
                    xch = px.tile([128, NCT * TCH], F32R, tag="xch",
                                  name="xch")
                    nc.gpsimd.dma_start(
                        xch.rearrange("p (n t) -> p n t", n=NCT),
                        xT[:, ch * TCH:(ch + 1) * TCH].rearrange(
                            "(n p) t -> p n t", p=128))
                    cs = pcs.tile([128, TCH], F32, tag="cos", name="cs")
                    sn = pcs.tile([128, TCH], F32, tag="sin", name="sn")
                    nc.sync.dma_start(cs[:], cosT[:, tloc:tloc + TCH])
                    nc.sync.dma_start(sn[:], sinT[:, tloc:tloc + TCH])
                    # q,k for each local head
                    for h in range(HPC):
                        p = b * HPC + h
                        for wsb, dst, nm in ((wq_sb, qT_sb, "q"),
                                             (wk_sb, kT_sb, "k")):
                            ps = pps.tile([128, TCH], F32, tag="pAqk",
                                          name="psA")
                            for ct in range(NCT):
                                nc.tensor.matmul(
                                    ps[:],
                                    wsb[:, ct * D + h * HD:
                                        ct * D + (h + 1) * HD],
                                    xch[:, ct * TCH:(ct + 1) * TCH],
                                    start=(ct == 0), stop=(ct == NCT - 1))
                            sl = dst[:, p * T + tloc: p * T + tloc + TCH]
                            tmp = pt.tile([128, TCH], F32, tag="rtmp",
                                          name="rtmp")
                            nc.scalar.copy(tmp[:], ps[:])
                            sw = pt.tile([128, TCH], F32, tag="rsw",
                                         name="rsw")
                            nc.sync.dma_start(sw[0:64, :], tmp[64:128, :])
                            nc.sync.dma_start(sw[64:128, :], tmp[0:64, :])
                            t1 = pt.tile([128, TCH], F32, tag="rt1",
                                         name="t1")
                            nc.vector.tensor_mul(t1[:], ps[:], cs[:])
                            t2 = pt.tile([128, TCH], F32, tag="rt2",
                                         name="t2")
                            nc.vector.tensor_mul(t2[:], sw[:], sn[:])
                            with nc.allow_low_precision(reason="f32r rope"):
                                nc.vector.tensor_add(sl, t1[:], t2[:])
                    # v for this chunk (all local heads at once)
                    for st in range(TCH // 128):
                        tt = (ch * TCH) // 128 + st   # global tile in [0,B*NTT)
                        ps = pps.tile([128, D], F32, tag="pAqk", name="psV")
                        for ct in range(NCT):
                            nc.tensor.matmul(
                                ps[:],
                                xch[:, ct * TCH + st * 128:
                                    ct * TCH + st * 128 + 128],
                                wv_sb[:, ct * D:(ct + 1) * D],
                                start=(ct == 0), stop=(ct == NCT - 1))
                        with nc.allow_low_precision(reason="f32r v evac"):
                            nc.scalar.copy(v_sb[:, tt * D:(tt + 1) * D],
                                           ps[:])

            # ============ Attention per (batch, head) =================
            # Software-pipelined emission: the S^T matmul for k-tile
            # kt+2 is issued before the den/PV matmuls of k-tile kt, so
            # the PE keeps streaming while ACT(exp)/DVE(mask) catch up.
            with tc.tile_pool(name="att_es", bufs=8) as pes, \
                 tc.tile_pool(name="att_o", bufs=3) as po, \
                 tc.tile_pool(name="att_ps", bufs=4, space="PSUM") as pas, \
                 tc.tile_pool(name="att_acc", bufs=2, space="PSUM") as paa, \
                 tc.tile_pool(name="att_msk", bufs=4) as pmk:
                for h in range(HPC):
                    for b in range(B):
                        p = b * HPC + h
                        for qj in range(NQC):
                            qsl = qT_sb[:, p * T + qj * 512:
                                        p * T + qj * 512 + 512]
                            kmax = ktmax(qj)
                            ps_den = paa.tile([1, 512], F32, tag="den", bufs=1,
                                              name="psden")
                            ps_o = paa.tile([128, 512], F32, tag="pvacc",
                                            name="pso")

                            def s_mm(kt):
                                ps_s = pas.tile([128, 512], F32, tag="s",
                                                name="pss")
                                nc.tensor.matmul(
                                    ps_s[:],
                                    kT_sb[:, p * T + kt * 128:
                                          p * T + kt * 128 + 128],
                                    qsl, start=True, stop=True)
                                return ps_s

                            s_tiles = {}
                            for kk in range(min(3, kmax)):
                                s_tiles[kk] = s_mm(kk)
                            for kt in range(kmax):
                                ps_s = s_tiles.pop(kt)
                                if mode == "masked":
                                    sm = pmk.tile([128, 512], F32, tag="sm",
                                                  name="sm")
                                    mt = pmk.tile([128, 512], F32, tag="mt",
                                                  name="mt")
                                    nc.sync.dma_start(
                                        mt[:],
                                        maskT[kt * 128:(kt + 1) * 128,
                                              qj * 512:(qj + 1) * 512])
                                    nc.vector.tensor_add(sm[:], ps_s[:],
                                                         mt[:])
                                    src = sm
                                else:
                                    src = ps_s
                                e_t = pes.tile([128, 512], F32R, tag="es",
                                               name="et")
                                with nc.allow_low_precision(reason="exp"):
                                    nc.scalar.activation(
                                        e_t[:], src[:], AF.Exp,
                                        scale=float(scale))
                                if mode == "causal" and kt >= 4 * qj:
                                    o = kt - 4 * qj
                                    em = pes.tile([128, 512], F32R,
                                                  tag="esm", name="em")
                                    with nc.allow_low_precision(reason="mask"):
                                        nc.vector.tensor_mul(
                                            em[:], e_t.bitcast(F32),
                                            bm_sb[:, 384 - 128 * o:
                                                  896 - 128 * o])
                                    e_t = em
                                if kt + 3 < kmax:
                                    s_tiles[kt + 3] = s_mm(kt + 3)
                                nc.tensor.matmul(
                                    ps_den[:], ones_sb[:], e_t[:],
                                    start=(kt == 0), stop=(kt == kmax - 1))
                                nc.tensor.matmul(
                                    ps_o[:],
                                    v_sb[:, (b * NTT + kt) * D + h * HD:
                                         (b * NTT + kt) * D + (h + 1) * HD],
                                    e_t[:],
                                    start=(kt == 0), stop=(kt == kmax - 1))
                            rd = po.tile([1, 512], F32R, tag="rd", name="rd")
                            with nc.allow_low_precision(reason="recip"):
                                nc.vector.reciprocal(rd[:], ps_den[:])
                            ps_b = pas.tile([128, 512], F32, tag="bc",
                                            bufs=1, name="psb")
                            nc.tensor.matmul(ps_b[:], onesr_sb[:], rd[:],
                                             start=True, stop=True)
                            o_tmp = po.tile([128, 512], F32, tag="otmp",
                                            name="otmp")
                            nc.vector.tensor_copy(o_tmp[:], ps_o[:])
                            o_sc = po.tile([128, 512], F32R, tag="osc",
                                           name="osc")
                            with nc.allow_low_precision(reason="scale"):
                                nc.vector.tensor_mul(o_sc[:], o_tmp[:],
                                                     ps_b[:])
                            # scatter the 512-wide q-chunk into shards
                            w = min(512, TO)
                            for s in range(512 // w):
                                t0 = qj * 512 + s * w    # global t in batch
                                shard = b * 4 + t0 // TO
                                nc.sync.dma_start(
                                    cc_in[h][shard * HD:(shard + 1) * HD,
                                             t0 % TO: t0 % TO + w],
                                    o_sc[:, s * w:(s + 1) * w])
                    # head h complete on both batches -> its AllToAll can
                    # overlap head h+1's attention
                    nc.gpsimd.collective_compute(
                        "AllToAll", mybir.AluOpType.bypass,
                        replica_groups=[list(range(N_CORES))],
                        ins=[cc_in[h].opt()], outs=[cc_out[h].opt()])

        # ============ Phase C: o_proj for this core's slice ===========
        with tc.tile_pool(name="phC_cc", bufs=1) as pcc, \
             tc.tile_pool(name="phC_w", bufs=3) as pcw, \
             tc.tile_pool(name="phC_y", bufs=4) as pcy, \
             tc.tile_pool(name="phC_ps", bufs=4, space="PSUM") as pcps:
            cc_sb = []
            for h in range(HPC):
                t = pcc.tile([128, N_CORES * TO], F32R, name=f"cc_sb{h}")
                nc.sync.dma_start(
                    t.rearrange("p (n t) -> p n t", n=N_CORES),
                    cc_out[h].opt().rearrange("(n p) t -> p n t", p=128))
                cc_sb.append(t)
            DW = 256
            for dj in range(C // DW):
                wo_sb = pcw.tile([128, NCT * DW], F32R, tag="wo", name="wo")
                nc.gpsimd.dma_start(
                    wo_sb.rearrange("p (n d) -> p n d", n=NCT),
                    woT[:, dj * DW:(dj + 1) * DW].rearrange(
                        "(n p) d -> p n d", p=128))
                for tt in range(TO // 128):
                    ps = pcps.tile([128, DW], F32, tag="pC", name="psC")
                    for ct in range(NCT):
                        # channel-tile ct = core (ct // HPC), head (ct % HPC)
                        i, hh = divmod(ct, HPC)
                        nc.tensor.matmul(
                            ps[:],
                            cc_sb[hh][:, i * TO + tt * 128:
                                      i * TO + tt * 128 + 128],
                            wo_sb[:, ct * DW:(ct + 1) * DW],
                            start=(ct == 0), stop=(ct == NCT - 1))
                    yt = pcy.tile([128, DW], F32, tag="yt", name="yt")
                    nc.scalar.copy(yt[:], ps[:])
                    nc.sync.dma_start(
                        y[tt * 128:(tt + 1) * 128, dj * DW:(dj + 1) * DW],
                        yt[:])

    nc.compile()
    return nc


_NC_CACHE = {}


def _get_nc(T, C, mode):
    key = (T, C, mode)
    if key not in _NC_CACHE:
        _NC_CACHE[key] = build_nc(T, C, mode)
    return _NC_CACHE[key]


def _detect_mode(mask):
    T = mask.shape[0]
    tri = np.tril(np.ones((T, T), dtype=bool))
    if not np.any(mask):
        return "full"
    if np.all(np.abs(mask[tri]) < 1e-6) and np.all(mask[~tri] < -1e8):
        return "causal"
    return "masked"


def kernel(x, mask, Wq, Wk, Wv, Wo):
    x = np.asarray(x)
    mask = np.asarray(mask)
    Bx, T, C = x.shape
    assert Bx == B
    HPC = C // HD // N_CORES
    TO = T // 4
    mode = _detect_mode(mask)
    nc = _get_nc(T, C, mode)

    cos, sin_signed = _rope_tables(T)
    xT2 = np.concatenate([x[0].T, x[1].T], axis=1)
    xT2 = np.ascontiguousarray(xT2)
    in_maps = []
    for core in range(N_CORES):
        hsl = slice(core * HPC * HD, (core + 1) * HPC * HD)
        m = {
            "xT": xT2,
            "wqT": np.ascontiguousarray(np.asarray(Wq)[hsl, :].T),
            "wkT": np.ascontiguousarray(np.asarray(Wk)[hsl, :].T),
            "wvT": np.ascontiguousarray(np.asarray(Wv)[hsl, :].T),
            "woT": np.ascontiguousarray(np.asarray(Wo).T),
            "cosT": cos, "sinT": sin_signed,
            "ones_in": np.ones((128, 1), np.float32),
            "onesr_in": np.ones((1, 128), np.float32),
        }
        if mode == "causal":
            m["bmask"] = _causal_binmask()
        elif mode == "masked":
            m["maskT"] = np.ascontiguousarray(mask.T) * np.float32(np.sqrt(HD))
        in_maps.append(m)

    res = bass_utils.run_bass_kernel_spmd(nc, in_maps,
                                          core_ids=list(range(N_CORES)))

    out = np.empty((B, T, C), np.float32)
    for core in range(N_CORES):
        b, g = divmod(core, 4)
        out[b, g * TO:(g + 1) * TO, :] = res.results[core]["y"]
    return out
